# revision 1
# baseline (speedup 1.0000x reference)
"""Trainium2 Bass kernel for nn_BoundarySuppressionWithSmoothing.

Full inputs: x [8,1,512,1024] f32, prediction [8,1,512,1024] int32.
Sharding: pure data parallel, image i -> core i.

Per-core algorithm (image I [512,1024], layout A: 4 row-chunks of [128,1024]):
  - boundary detection via exp-encoded morphology on PE + ACT (exp/ln-free
    product compare), masks m3..m0 via a mask-carried dilation chain
  - 4 iterations of masked 3x3 box average with replication padding
  - separable dilated 7x7 Gaussian (dilation 6) via PE banded matmuls
"""
import math
import sys
from contextlib import ExitStack

import numpy as np

sys.path.insert(0, '/opt/trn_rl_repo')

import concourse.bass as bass  # noqa: E402
import concourse.bacc as bacc  # noqa: E402
import concourse.tile as tile  # noqa: E402
from concourse import mybir  # noqa: E402

P = 128
W = 1024
H = 512
CH = 4          # row chunks
B = 8           # batch == cores
ALPHA = 4.6     # morphology exp-encoding scale
PTHR = float(np.exp(4.2))   # product threshold for boundary test
DT = mybir.dt
AF = mybir.ActivationFunctionType
OP = mybir.AluOpType

USE_DIVIDE = True   # t = Y / n via TT divide; else reciprocal+mult


# ---------------------------------------------------------------- weights ---
def _gauss1d():
    size, sigma = 7, 1.0
    u = np.exp(-((np.arange(size) - 3.0) ** 2) / (2 * sigma ** 2))
    # 2D reference kernel is outer(u,u)/sum => separable 1D = u/sum(u)
    return (u / u.sum()).astype(np.float64)


def _round_fp32r(a):
    """Round fp32 array to fp32r (11 explicit mantissa bits) on host."""
    u = a.astype(np.float32).view(np.uint32).astype(np.uint64)
    u = (u + 0x800) & 0xFFFFF000
    return (u & 0xFFFFFFFF).astype(np.uint32).view(np.float32)


def build_host_consts():
    """All constant weight matrices, as one dict of fp32 arrays [128,x]."""
    c = {}
    tri = np.zeros((P, P), np.float32)
    for k in range(P):
        for d in (-1, 0, 1):
            if 0 <= k + d < P:
                tri[k, k + d] = 1.0   # lhsT[k,m]: out m from in k, |k-m|<=1
    c['T_mid'] = tri
    t_top = tri.copy(); t_top[0, 0] = 2.0
    c['T_top'] = t_top
    t_bot = tri.copy(); t_bot[P - 1, P - 1] = 2.0
    c['T_bot'] = t_bot
    t_up = np.zeros((P, P), np.float32); t_up[P - 1, 0] = 1.0
    c['T_up'] = t_up
    t_dn = np.zeros((P, P), np.float32); t_dn[0, P - 1] = 1.0
    c['T_dn'] = t_dn
    c['I'] = np.eye(P, dtype=np.float32)
    bvec = np.zeros((P, P), np.float32)
    bvec[:, 0] = -4.0; bvec[0, 0] = -3.0      # bv_top
    bvec[:, 1] = -4.0; bvec[P - 1, 1] = -3.0  # bv_bot
    c['BVEC'] = bvec

    g = _gauss1d()
    for j in range(7):
        c[f'G{j}'] = _round_fp32r(np.eye(P, dtype=np.float32) * g[j])
    # vertical gaussian: Wv[R,S] = sum_j g[j] [clamp(R+6(j-3),0,H-1)==S]
    Wv = np.zeros((H, H), np.float64)
    for R in range(H):
        for j in range(7):
            S = min(max(R + 6 * (j - 3), 0), H - 1)
            Wv[R, S] += g[j]
    for c_dst in range(CH):
        for c_src in range(CH):
            if abs(c_dst - c_src) > 1:
                continue
            blk = Wv[c_dst * P:(c_dst + 1) * P, c_src * P:(c_src + 1) * P]
            if not blk.any():
                continue
            # lhsT[k,m] = Wv[dst=128c+m, src=128c'+k]
            c[f'B_{c_dst}_{c_src}'] = _round_fp32r(
                np.ascontiguousarray(blk.T).astype(np.float32))
    return c


# ----------------------------------------------------------------- kernel ---
def build_kernel(ctx: ExitStack, tc: "tile.TileContext", outs, ins):
    nc = tc.nc
    y = outs[0]                       # [512,1024] f32 DRAM
    x, pred, wpack = ins              # wpack [128, NW*128] f32 DRAM

    consts = build_host_consts()
    wnames = sorted(consts.keys())

    sb = ctx.enter_context(tc.tile_pool(name="sb", bufs=1))
    sbR = ctx.enter_context(tc.tile_pool(name="sbR", bufs=2))
    wpool = ctx.enter_context(tc.tile_pool(name="wp", bufs=1))
    psB = ctx.enter_context(tc.tile_pool(name="psB", bufs=2, space="PSUM"))
    psY = ctx.enter_context(tc.tile_pool(name="psY", bufs=4, space="PSUM"))

    # ---- load + prepare weights ----
    wstage = sb.tile([P, len(wnames) * P], DT.float32, tag="wstage")
    nc.sync.dma_start(wstage[:], wpack[:, :len(wnames) * P])
    wt = {}
    BF16_W = {'T_mid', 'T_top', 'T_bot', 'T_up', 'T_dn', 'I'}
    for i, name in enumerate(wnames):
        if name == 'BVEC':
            continue
        src = wstage[:, i * P:(i + 1) * P]
        dt_w = DT.bfloat16 if name in BF16_W else DT.float32r
        t = wpool.tile([P, P], dt_w, name=f"w_{name}", tag=f"w_{name}")
        nc.vector.tensor_copy(t[:], src)
        wt[name] = t
    # fp32r variants of vertical matrices for the value path
    for name in ('T_mid', 'T_top', 'T_bot', 'T_up', 'T_dn'):
        t = wpool.tile([P, P], DT.float32r, name=f"wr_{name}", tag=f"wr_{name}")
        i = wnames.index(name)
        nc.vector.tensor_copy(t[:], wstage[:, i * P:(i + 1) * P])
        wt['R' + name[1:]] = t

    def TRv(c):
        return wt['T_top'] if c == 0 else (wt['T_bot'] if c == CH - 1 else wt['T_mid'])

    def Rv(c):
        return wt['R_top'] if c == 0 else (wt['R_bot'] if c == CH - 1 else wt['R_mid'])

    # ---- const bias vectors ----
    def make_const(val, tag):
        t = sb.tile([P, 1], DT.float32, tag=tag)
        nc.vector.memset(t[:], val)
        return t

    b_enc_max = make_const(-9.0 * ALPHA, "b_enc_max")
    b_enc_min = make_const(+9.0 * ALPHA, "b_enc_min")
    bv_mid = make_const(-4.0, "bv_mid")
    ib = wnames.index('BVEC')
    bv_top = sb.tile([P, 1], DT.float32, name="bv_top", tag="bv_top")
    nc.vector.tensor_copy(bv_top[:], wstage[:, ib * P:ib * P + 1])
    bv_bot = sb.tile([P, 1], DT.float32, name="bv_bot", tag="bv_bot")
    nc.vector.tensor_copy(bv_bot[:], wstage[:, ib * P + 1:ib * P + 2])
    one_c = make_const(1.0, "one_c")

    def bv(c):
        return bv_top if c == 0 else (bv_bot if c == CH - 1 else bv_mid)

    # ---- persistent image buffers ----
    lab = [sb.tile([P, W], DT.int32, name=f"lab{c}", tag=f"lab{c}") for c in range(CH)]
    OA = [sb.tile([P, W], DT.float32, name=f"OA{c}", tag=f"OA{c}") for c in range(CH)]
    OB = [sb.tile([P, W], DT.float32, name=f"OB{c}", tag=f"OB{c}") for c in range(CH)]
    for c in range(CH):
        nc.sync.dma_start(OA[c][:], x[c * P:(c + 1) * P, :])
        nc.sync.dma_start(lab[c][:], pred[c * P:(c + 1) * P, :])

    GW = W + 2

    def gtile(tag, dtype, guard_val, pool=sb):
        ts = [pool.tile([P, GW], dtype, name=f"{tag}{c}", tag=f"{tag}{c}") for c in range(CH)]
        for c in range(CH):
            for ap in (ts[c][:, 0:1], ts[c][:, GW - 1:GW]):
                if dtype == DT.float32r:
                    ap = ap.bitcast(DT.float32)
                nc.vector.memset(ap, guard_val)
        return ts

    Emax = gtile("Emax", DT.bfloat16, 0.0)
    Emin = gtile("Emin", DT.bfloat16, 0.0)
    m = [gtile(f"m{i}_", DT.bfloat16, 1.0) for i in range(4)]
    xm = gtile("xm", DT.float32r, 0.0)
    HN = [sb.tile([P, W], DT.bfloat16, name=f"HN{c}", tag=f"HMa{c}") for c in range(CH)]
    HMa = [sb.tile([P, W], DT.bfloat16, name=f"HMa{c}", tag=f"HMa{c}") for c in range(CH)]
    hlr = [sb.tile([P, W], DT.float32r, name=f"hlr{c}", tag=f"hlr{c}") for c in range(CH)]

    def data(t):
        return t[:, 1:W + 1]

    def shl(t):
        return t[:, 0:W]

    def shr(t):
        return t[:, 2:W + 2]

    def mm_group(pt, pairs):
        # split into N=512 sub-matmuls (PSUM bank limit); weight-major order
        # so consecutive matmuls share the stationary operand (fewer LDW).
        n = pt.shape[1]
        halves = list(range(0, n, 512))
        for i, (lhsT, rhs) in enumerate(pairs):
            for h0 in halves:
                nc.tensor.matmul(pt[:, h0:h0 + 512], lhsT,
                                 rhs[:, h0:h0 + 512], start=(i == 0),
                                 stop=(i == len(pairs) - 1))

    # ================= Phase M: encode + boundary masks ===================
    for c in range(CH):
        nc.scalar.activation(data(Emax[c]), lab[c][:], AF.Exp,
                             bias=b_enc_max[:], scale=ALPHA)
        nc.scalar.activation(data(Emin[c]), lab[c][:], AF.Exp,
                             bias=b_enc_min[:], scale=-ALPHA)
    for c in range(CH):
        nc.vector.tensor_tensor(HN[c][:], shl(Emin[c]), shr(Emin[c]), op=OP.add)
        nc.vector.tensor_tensor(HN[c][:], HN[c][:], data(Emin[c]), op=OP.add)
    for c in range(CH):
        p1 = psB.tile([P, W], DT.float32, name="pS1", tag="psb")
        pairs = [(wt['T_mid'][:], data(Emax[c])),
                 (wt['I'][:], shl(Emax[c])),
                 (wt['I'][:], shr(Emax[c]))]
        if c > 0:
            pairs.append((wt['T_up'][:], data(Emax[c - 1])))
        if c < CH - 1:
            pairs.append((wt['T_dn'][:], data(Emax[c + 1])))
        mm_group(p1[:], pairs)
        sc1 = sbR.tile([P, W], DT.bfloat16, name="sc1", tag="nb")
        nc.scalar.copy(sc1[:], p1[:])

        p2 = psB.tile([P, W], DT.float32, name="pS2", tag="psb")
        pairs = [(wt['T_mid'][:], HN[c][:])]
        if c > 0:
            pairs.append((wt['T_up'][:], HN[c - 1][:]))
        if c < CH - 1:
            pairs.append((wt['T_dn'][:], HN[c + 1][:]))
        mm_group(p2[:], pairs)
        pb = sbR.tile([P, W], DT.bfloat16, name="pb", tag="zt")
        nc.vector.tensor_tensor(pb[:], sc1[:], p2[:], op=OP.mult)
        nc.vector.tensor_scalar(data(m[3][c]), pb[:], PTHR, None, op0=OP.is_lt)

    # ================= Chain: m3 -> m2 -> m1 -> m0 ========================
    for k in range(3):
        mp, mn = m[3 - k], m[2 - k]
        for c in range(CH):
            ps = psB.tile([P, W], DT.float32, name="pCh", tag="psb")
            pairs = [(wt['T_mid'][:], data(mp[c])),
                     (wt['I'][:], shl(mp[c])),
                     (wt['I'][:], shr(mp[c]))]
            if c > 0:
                pairs.append((wt['T_up'][:], data(mp[c - 1])))
            if c < CH - 1:
                pairs.append((wt['T_dn'][:], data(mp[c + 1])))
            mm_group(ps[:], pairs)
            nc.scalar.activation(data(mn[c]), ps[:], AF.Relu, bias=bv(c)[:],
                                 scale=1.0)

    # ================= U loop =============================================
    cur, nxt = OA, OB
    for it in range(4):
        mi = m[it]
        for c in range(CH):
            nc.gpsimd.tensor_tensor(xm[c][:, 1:W + 1], cur[c][:], data(mi[c]),
                                    op=OP.mult)
            nc.gpsimd.tensor_tensor(HMa[c][:], shl(mi[c]), shr(mi[c]), op=OP.add)
        for c in range(CH):
            # HMa := full hsum3_rep(m) = mL + mR + m, with edge fixes
            nc.vector.tensor_tensor(HMa[c][:], HMa[c][:], data(mi[c]), op=OP.add)
            nc.vector.tensor_scalar(HMa[c][:, 0:1], mi[c][:, 1:2], 2.0, None,
                                    op0=OP.mult)
            nc.vector.tensor_tensor(HMa[c][:, 0:1], HMa[c][:, 0:1],
                                    mi[c][:, 2:3], op=OP.add)
            nc.vector.tensor_scalar(HMa[c][:, W - 1:W], mi[c][:, W:W + 1], 2.0,
                                    None, op0=OP.mult)
            nc.vector.tensor_tensor(HMa[c][:, W - 1:W], HMa[c][:, W - 1:W],
                                    mi[c][:, W - 1:W], op=OP.add)
            # hlr := xmL + xmR (DVE), edge fixes, then SH := hlr + xm (gpsimd)
            nc.vector.tensor_tensor(hlr[c][:], shl(xm[c]), shr(xm[c]), op=OP.add)
            nc.vector.tensor_tensor(hlr[c][:, 0:1], hlr[c][:, 0:1],
                                    xm[c][:, 1:2], op=OP.add)
            nc.vector.tensor_tensor(hlr[c][:, W - 1:W], hlr[c][:, W - 1:W],
                                    xm[c][:, W:W + 1], op=OP.add)
        for c in range(CH):
            nc.gpsimd.tensor_tensor(hlr[c][:], hlr[c][:], xm[c][:, 1:W + 1],
                                    op=OP.add)
        for c in range(CH):
            pn = psB.tile([P, W], DT.float32, name="pN", tag="psb")
            pairs = [(TRv(c)[:], HMa[c][:])]
            if c > 0:
                pairs.append((wt['T_up'][:], HMa[c - 1][:]))
            if c < CH - 1:
                pairs.append((wt['T_dn'][:], HMa[c + 1][:]))
            mm_group(pn[:], pairs)
            zt = sbR.tile([P, W], DT.bfloat16, name="zt", tag="zt")
            nc.scalar.activation(zt[:], pn[:], AF.Relu, bias=one_c[:],
                                 scale=-1.0)
            nb = sbR.tile([P, W], DT.float32, name="nb", tag="nb")
            nc.vector.reciprocal(nb[:], pn[:])
            Mk = sbR.tile([P, W], DT.int16, name="Mk", tag="Mk")
            nc.vector.tensor_tensor(Mk[:], data(mi[c]), zt[:], op=OP.add)

            for h in range(2):
                s = slice(h * 512, (h + 1) * 512)
                sg = slice(1 + h * 512, 1 + (h + 1) * 512)
                pyt = psY.tile([P, 512], DT.float32, name="pY", tag="psy")
                pairs = [(Rv(c)[:], hlr[c][:, s])]
                if c > 0:
                    pairs.append((wt['R_up'][:], hlr[c - 1][:, s]))
                if c < CH - 1:
                    pairs.append((wt['R_dn'][:], hlr[c + 1][:, s]))
                mm_group(pyt[:], pairs)
                nc.vector.tensor_tensor(nxt[c][:, s], pyt[:], nb[:, s],
                                        op=OP.mult)
            nc.vector.copy_predicated(nxt[c][:], Mk[:], cur[c][:])
        cur, nxt = nxt, cur

    # ================= Gaussian ==========================================
    GA = 18
    gs = [sb.tile([P, W + 2 * GA], DT.float32r, name=f"gs{c}", tag=f"lab{c}")
          for c in range(CH)]
    hg = [sb.tile([P, W], DT.float32r, name=f"Emin{c}", tag=f"Emin{c}") for c in range(CH)]
    yo = [sb.tile([P, W], DT.float32, name=f"Emax{c}", tag=f"Emax{c}") for c in range(CH)]
    for c in range(CH):
        nc.vector.tensor_copy(gs[c][:, GA:GA + W], cur[c][:])
        nc.vector.tensor_copy(gs[c][:, 0:GA],
                              cur[c][:, 0:1].to_broadcast((P, GA)))
        nc.vector.tensor_copy(gs[c][:, GA + W:],
                              cur[c][:, W - 1:W].to_broadcast((P, GA)))
    for c in range(CH):
        for h in range(2):
            ph = psY.tile([P, 512], DT.float32, name="pH", tag="psy")
            for j in range(7):
                off = GA + 6 * (j - 3) + h * 512
                nc.tensor.matmul(ph[:], wt[f'G{j}'][:], gs[c][:, off:off + 512],
                                 start=(j == 0), stop=(j == 6))
            nc.scalar.copy(hg[c][:, h * 512:(h + 1) * 512], ph[:])
    for c in range(CH):
        for h in range(2):
            s = slice(h * 512, (h + 1) * 512)
            pv = psY.tile([P, 512], DT.float32, name="pV", tag="psy")
            srcs = [cc for cc in range(CH) if f'B_{c}_{cc}' in wt]
            for i, cc in enumerate(srcs):
                nc.tensor.matmul(pv[:], wt[f'B_{c}_{cc}'][:], hg[cc][:, s],
                                 start=(i == 0), stop=(i == len(srcs) - 1))
            nc.scalar.copy(yo[c][:, s], pv[:])
    for c in range(CH):
        nc.sync.dma_start(y[c * P:(c + 1) * P, :], yo[c][:])


# ------------------------------------------------------------ host driver ---
_CACHE = {}


def _build_program():
    if 'nc' in _CACHE:
        return _CACHE['nc'], _CACHE['wpack']
    consts = build_host_consts()
    wnames = sorted(consts.keys())
    wpack = np.zeros((P, len(wnames) * P), np.float32)
    for i, n in enumerate(wnames):
        wpack[:, i * P:(i + 1) * P] = consts[n]

    nc = bacc.Bacc("TRN2", target_bir_lowering=False, debug=False,
                   num_devices=B)
    x_d = nc.dram_tensor("x", [H, W], DT.float32, kind="ExternalInput").ap()
    p_d = nc.dram_tensor("prediction", [H, W], DT.int32,
                         kind="ExternalInput").ap()
    w_d = nc.dram_tensor("wpack", list(wpack.shape), DT.float32,
                         kind="ExternalInput").ap()
    y_d = nc.dram_tensor("y", [H, W], DT.float32, kind="ExternalOutput").ap()
    with tile.TileContext(nc) as tc:
        with ExitStack() as ctx:
            build_kernel(ctx, tc, [y_d], [x_d, p_d, w_d])
    nc.compile()
    _CACHE['nc'] = nc
    _CACHE['wpack'] = wpack
    return nc, wpack


def _run(x, prediction, trace=False):
    from concourse.bass_utils import run_bass_kernel_spmd
    nc, wpack = _build_program()
    in_maps = []
    for i in range(B):
        in_maps.append({
            "x": np.ascontiguousarray(x[i, 0]).astype(np.float32),
            "prediction": np.ascontiguousarray(prediction[i, 0]).astype(np.int32),
            "wpack": wpack,
        })
    res = run_bass_kernel_spmd(nc, in_maps, core_ids=list(range(B)),
                               trace=trace)
    if trace:
        print(f"HW exec time: {res.exec_time_ns} ns "
              f"(mean {res.mean_exec_time_ns} ns, "
              f"slowest core {res.max_exec_time_core_id})")
        if res.instructions_and_trace:
            print("trace:", res.instructions_and_trace[1])
    out = np.stack([res.results[i]["y"] for i in range(B)], axis=0)
    return out[:, None, :, :].astype(np.float32)


def kernel(x: np.ndarray, prediction: np.ndarray) -> np.ndarray:
    return _run(x, prediction, trace=False)


def kernel_traced(x, prediction, trace=True):
    return _run(x, prediction, trace=trace)


if __name__ == "__main__":
    xs = np.random.randn(B, 1, H, W).astype(np.float32)
    ps = np.random.randint(0, 19, size=(B, 1, H, W)).astype(np.int32)
    print(kernel(xs, ps).shape)



# revision 2
# speedup vs baseline: 339.9763x; 339.9763x over previous
"""Trainium2 Bass kernel for nn_BoundarySuppressionWithSmoothing.

Full inputs: x [8,1,512,1024] f32, prediction [8,1,512,1024] int32.
Sharding: pure data parallel, image i -> core i.

Per-core algorithm (image I [512,1024], layout A: 4 row-chunks of [128,1024]):
  - boundary detection via exp-encoded morphology on PE + ACT (exp/ln-free
    product compare), masks m3..m0 via a mask-carried dilation chain
  - 4 iterations of masked 3x3 box average with replication padding
  - separable dilated 7x7 Gaussian (dilation 6) via PE banded matmuls
"""
import math
import sys
from contextlib import ExitStack

import numpy as np

sys.path.insert(0, '/opt/trn_rl_repo')

import concourse.bass as bass  # noqa: E402
import concourse.bacc as bacc  # noqa: E402
import concourse.tile as tile  # noqa: E402
from concourse import mybir  # noqa: E402

P = 128
W = 1024
H = 512
CH = 4          # row chunks
B = 8           # batch == cores
ALPHA = 4.6     # morphology exp-encoding scale
PTHR = float(np.exp(4.2))   # product threshold for boundary test
DT = mybir.dt
AF = mybir.ActivationFunctionType
OP = mybir.AluOpType

USE_DIVIDE = True   # t = Y / n via TT divide; else reciprocal+mult


# ---------------------------------------------------------------- weights ---
def _gauss1d():
    size, sigma = 7, 1.0
    u = np.exp(-((np.arange(size) - 3.0) ** 2) / (2 * sigma ** 2))
    # 2D reference kernel is outer(u,u)/sum => separable 1D = u/sum(u)
    return (u / u.sum()).astype(np.float64)


def _round_fp32r(a):
    """Round fp32 array to fp32r (11 explicit mantissa bits) on host."""
    u = a.astype(np.float32).view(np.uint32).astype(np.uint64)
    u = (u + 0x800) & 0xFFFFF000
    return (u & 0xFFFFFFFF).astype(np.uint32).view(np.float32)


def build_host_consts():
    """All constant weight matrices, as one dict of fp32 arrays [128,x]."""
    c = {}
    tri = np.zeros((P, P), np.float32)
    for k in range(P):
        for d in (-1, 0, 1):
            if 0 <= k + d < P:
                tri[k, k + d] = 1.0   # lhsT[k,m]: out m from in k, |k-m|<=1
    c['T_mid'] = tri
    t_top = tri.copy(); t_top[0, 0] = 2.0
    c['T_top'] = t_top
    t_bot = tri.copy(); t_bot[P - 1, P - 1] = 2.0
    c['T_bot'] = t_bot
    t_up = np.zeros((P, P), np.float32); t_up[P - 1, 0] = 1.0
    c['T_up'] = t_up
    t_dn = np.zeros((P, P), np.float32); t_dn[0, P - 1] = 1.0
    c['T_dn'] = t_dn
    c['I'] = np.eye(P, dtype=np.float32)
    bvec = np.zeros((P, P), np.float32)
    bvec[:, 0] = -4.0; bvec[0, 0] = -3.0      # bv_top
    bvec[:, 1] = -4.0; bvec[P - 1, 1] = -3.0  # bv_bot
    c['BVEC'] = bvec

    g = _gauss1d()
    for j in range(7):
        c[f'G{j}'] = _round_fp32r(np.eye(P, dtype=np.float32) * g[j])
    # vertical gaussian: Wv[R,S] = sum_j g[j] [clamp(R+6(j-3),0,H-1)==S]
    Wv = np.zeros((H, H), np.float64)
    for R in range(H):
        for j in range(7):
            S = min(max(R + 6 * (j - 3), 0), H - 1)
            Wv[R, S] += g[j]
    for c_dst in range(CH):
        for c_src in range(CH):
            if abs(c_dst - c_src) > 1:
                continue
            blk = Wv[c_dst * P:(c_dst + 1) * P, c_src * P:(c_src + 1) * P]
            if not blk.any():
                continue
            # lhsT[k,m] = Wv[dst=128c+m, src=128c'+k]
            c[f'B_{c_dst}_{c_src}'] = _round_fp32r(
                np.ascontiguousarray(blk.T).astype(np.float32))
    return c


# ----------------------------------------------------------------- kernel ---
def build_kernel(ctx: ExitStack, tc: "tile.TileContext", outs, ins):
    nc = tc.nc
    y = outs[0]                       # [512,1024] f32 DRAM
    x, pred, wpack = ins              # wpack [128, NW*128] f32 DRAM

    consts = build_host_consts()
    wnames = sorted(consts.keys())

    sb = ctx.enter_context(tc.tile_pool(name="sb", bufs=1))
    sbR = ctx.enter_context(tc.tile_pool(name="sbR", bufs=2))
    wpool = ctx.enter_context(tc.tile_pool(name="wp", bufs=1))
    psB = ctx.enter_context(tc.tile_pool(name="psB", bufs=2, space="PSUM"))
    psY = ctx.enter_context(tc.tile_pool(name="psY", bufs=4, space="PSUM"))

    # ---- load + prepare weights ----
    wstage = sb.tile([P, len(wnames) * P], DT.float32, tag="wstage")
    nc.sync.dma_start(wstage[:], wpack[:, :len(wnames) * P])
    wt = {}
    BF16_W = {'T_mid', 'T_top', 'T_bot', 'T_up', 'T_dn', 'I'}
    for i, name in enumerate(wnames):
        if name == 'BVEC':
            continue
        src = wstage[:, i * P:(i + 1) * P]
        dt_w = DT.bfloat16 if name in BF16_W else DT.float32r
        t = wpool.tile([P, P], dt_w, name=f"w_{name}", tag=f"w_{name}")
        nc.vector.tensor_copy(t[:], src)
        wt[name] = t
    # fp32r variants of vertical matrices for the value path
    for name in ('T_mid', 'T_top', 'T_bot', 'T_up', 'T_dn'):
        t = wpool.tile([P, P], DT.float32r, name=f"wr_{name}", tag=f"wr_{name}")
        i = wnames.index(name)
        nc.vector.tensor_copy(t[:], wstage[:, i * P:(i + 1) * P])
        wt['R' + name[1:]] = t

    def TRv(c):
        return wt['T_top'] if c == 0 else (wt['T_bot'] if c == CH - 1 else wt['T_mid'])

    def Rv(c):
        return wt['R_top'] if c == 0 else (wt['R_bot'] if c == CH - 1 else wt['R_mid'])

    # ---- const bias vectors ----
    def make_const(val, tag):
        t = sb.tile([P, 1], DT.float32, tag=tag)
        nc.vector.memset(t[:], val)
        return t

    b_enc_max = make_const(-9.0 * ALPHA, "b_enc_max")
    b_enc_min = make_const(+9.0 * ALPHA, "b_enc_min")
    bv_mid = make_const(-4.0, "bv_mid")
    ib = wnames.index('BVEC')
    bv_top = sb.tile([P, 1], DT.float32, name="bv_top", tag="bv_top")
    nc.vector.tensor_copy(bv_top[:], wstage[:, ib * P:ib * P + 1])
    bv_bot = sb.tile([P, 1], DT.float32, name="bv_bot", tag="bv_bot")
    nc.vector.tensor_copy(bv_bot[:], wstage[:, ib * P + 1:ib * P + 2])
    one_c = make_const(1.0, "one_c")

    def bv(c):
        return bv_top if c == 0 else (bv_bot if c == CH - 1 else bv_mid)

    # ---- persistent image buffers ----
    lab = [sb.tile([P, W], DT.int32, name=f"lab{c}", tag=f"lab{c}") for c in range(CH)]
    OA = [sb.tile([P, W], DT.float32, name=f"OA{c}", tag=f"OA{c}") for c in range(CH)]
    OB = [sb.tile([P, W], DT.float32, name=f"OB{c}", tag=f"OB{c}") for c in range(CH)]
    for c in range(CH):
        nc.sync.dma_start(OA[c][:], x[c * P:(c + 1) * P, :])
        nc.sync.dma_start(lab[c][:], pred[c * P:(c + 1) * P, :])

    GW = W + 2

    def gtile(tag, dtype, guard_val, pool=sb):
        ts = [pool.tile([P, GW], dtype, name=f"{tag}{c}", tag=f"{tag}{c}") for c in range(CH)]
        for c in range(CH):
            for ap in (ts[c][:, 0:1], ts[c][:, GW - 1:GW]):
                if dtype == DT.float32r:
                    ap = ap.bitcast(DT.float32)
                nc.vector.memset(ap, guard_val)
        return ts

    Emax = gtile("Emax", DT.bfloat16, 0.0)
    Emin = gtile("Emin", DT.bfloat16, 0.0)
    m = [gtile(f"m{i}_", DT.bfloat16, 1.0) for i in range(4)]
    xm = gtile("xm", DT.float32r, 0.0)
    HN = [sb.tile([P, W], DT.bfloat16, name=f"HN{c}", tag=f"HMa{c}") for c in range(CH)]
    HMa = [sb.tile([P, W], DT.bfloat16, name=f"HMa{c}", tag=f"HMa{c}") for c in range(CH)]
    hlr = [sb.tile([P, W], DT.float32r, name=f"hlr{c}", tag=f"hlr{c}") for c in range(CH)]

    def data(t):
        return t[:, 1:W + 1]

    def shl(t):
        return t[:, 0:W]

    def shr(t):
        return t[:, 2:W + 2]

    def mm_group(pt, pairs):
        # split into N=512 sub-matmuls (PSUM bank limit); weight-major order
        # so consecutive matmuls share the stationary operand (fewer LDW).
        n = pt.shape[1]
        halves = list(range(0, n, 512))
        for i, (lhsT, rhs) in enumerate(pairs):
            for h0 in halves:
                nc.tensor.matmul(pt[:, h0:h0 + 512], lhsT,
                                 rhs[:, h0:h0 + 512], start=(i == 0),
                                 stop=(i == len(pairs) - 1))

    # ================= Phase M: encode + boundary masks ===================
    for c in range(CH):
        nc.scalar.activation(data(Emax[c]), lab[c][:], AF.Exp,
                             bias=b_enc_max[:], scale=ALPHA)
        nc.scalar.activation(data(Emin[c]), lab[c][:], AF.Exp,
                             bias=b_enc_min[:], scale=-ALPHA)
    for c in range(CH):
        nc.vector.tensor_tensor(HN[c][:], shl(Emin[c]), shr(Emin[c]), op=OP.add)
        nc.vector.tensor_tensor(HN[c][:], HN[c][:], data(Emin[c]), op=OP.add)
    for c in range(CH):
        p1 = psB.tile([P, W], DT.float32, name="pS1", tag="psb")
        pairs = [(wt['T_mid'][:], data(Emax[c])),
                 (wt['I'][:], shl(Emax[c])),
                 (wt['I'][:], shr(Emax[c]))]
        if c > 0:
            pairs.append((wt['T_up'][:], data(Emax[c - 1])))
        if c < CH - 1:
            pairs.append((wt['T_dn'][:], data(Emax[c + 1])))
        mm_group(p1[:], pairs)
        sc1 = sbR.tile([P, W], DT.bfloat16, name="sc1", tag="nb")
        nc.scalar.copy(sc1[:], p1[:])

        p2 = psB.tile([P, W], DT.float32, name="pS2", tag="psb")
        pairs = [(wt['T_mid'][:], HN[c][:])]
        if c > 0:
            pairs.append((wt['T_up'][:], HN[c - 1][:]))
        if c < CH - 1:
            pairs.append((wt['T_dn'][:], HN[c + 1][:]))
        mm_group(p2[:], pairs)
        pb = sbR.tile([P, W], DT.bfloat16, name="pb", tag="zt")
        nc.vector.tensor_tensor(pb[:], sc1[:], p2[:], op=OP.mult)
        nc.vector.tensor_scalar(data(m[3][c]), pb[:], PTHR, None, op0=OP.is_lt)

    # ================= Chain: m3 -> m2 -> m1 -> m0 ========================
    for k in range(3):
        mp, mn = m[3 - k], m[2 - k]
        for c in range(CH):
            ps = psB.tile([P, W], DT.float32, name="pCh", tag="psb")
            pairs = [(wt['T_mid'][:], data(mp[c])),
                     (wt['I'][:], shl(mp[c])),
                     (wt['I'][:], shr(mp[c]))]
            if c > 0:
                pairs.append((wt['T_up'][:], data(mp[c - 1])))
            if c < CH - 1:
                pairs.append((wt['T_dn'][:], data(mp[c + 1])))
            mm_group(ps[:], pairs)
            nc.scalar.activation(data(mn[c]), ps[:], AF.Relu, bias=bv(c)[:],
                                 scale=1.0)

    # ================= U loop =============================================
    cur, nxt = OA, OB
    for it in range(4):
        mi = m[it]
        for c in range(CH):
            nc.gpsimd.tensor_tensor(xm[c][:, 1:W + 1], cur[c][:], data(mi[c]),
                                    op=OP.mult)
            nc.gpsimd.tensor_tensor(HMa[c][:], shl(mi[c]), shr(mi[c]), op=OP.add)
        for c in range(CH):
            # HMa := full hsum3_rep(m) = mL + mR + m, with edge fixes
            nc.vector.tensor_tensor(HMa[c][:], HMa[c][:], data(mi[c]), op=OP.add)
            nc.vector.tensor_scalar(HMa[c][:, 0:1], mi[c][:, 1:2], 2.0, None,
                                    op0=OP.mult)
            nc.vector.tensor_tensor(HMa[c][:, 0:1], HMa[c][:, 0:1],
                                    mi[c][:, 2:3], op=OP.add)
            nc.vector.tensor_scalar(HMa[c][:, W - 1:W], mi[c][:, W:W + 1], 2.0,
                                    None, op0=OP.mult)
            nc.vector.tensor_tensor(HMa[c][:, W - 1:W], HMa[c][:, W - 1:W],
                                    mi[c][:, W - 1:W], op=OP.add)
            # hlr := xmL + xmR (DVE), edge fixes, then SH := hlr + xm (gpsimd)
            nc.vector.tensor_tensor(hlr[c][:], shl(xm[c]), shr(xm[c]), op=OP.add)
            nc.vector.tensor_tensor(hlr[c][:, 0:1], hlr[c][:, 0:1],
                                    xm[c][:, 1:2], op=OP.add)
            nc.vector.tensor_tensor(hlr[c][:, W - 1:W], hlr[c][:, W - 1:W],
                                    xm[c][:, W:W + 1], op=OP.add)
        for c in range(CH):
            nc.gpsimd.tensor_tensor(hlr[c][:], hlr[c][:], xm[c][:, 1:W + 1],
                                    op=OP.add)
        for c in range(CH):
            pn = psB.tile([P, W], DT.float32, name="pN", tag="psb")
            pairs = [(TRv(c)[:], HMa[c][:])]
            if c > 0:
                pairs.append((wt['T_up'][:], HMa[c - 1][:]))
            if c < CH - 1:
                pairs.append((wt['T_dn'][:], HMa[c + 1][:]))
            mm_group(pn[:], pairs)
            zt = sbR.tile([P, W], DT.bfloat16, name="zt", tag="zt")
            nc.scalar.activation(zt[:], pn[:], AF.Relu, bias=one_c[:],
                                 scale=-1.0)
            nb = sbR.tile([P, W], DT.float32, name="nb", tag="nb")
            nc.vector.reciprocal(nb[:], pn[:])
            Mk = sbR.tile([P, W], DT.int16, name="Mk", tag="Mk")
            nc.vector.tensor_tensor(Mk[:], data(mi[c]), zt[:], op=OP.add)

            for h in range(2):
                s = slice(h * 512, (h + 1) * 512)
                sg = slice(1 + h * 512, 1 + (h + 1) * 512)
                pyt = psY.tile([P, 512], DT.float32, name="pY", tag="psy")
                pairs = [(Rv(c)[:], hlr[c][:, s])]
                if c > 0:
                    pairs.append((wt['R_up'][:], hlr[c - 1][:, s]))
                if c < CH - 1:
                    pairs.append((wt['R_dn'][:], hlr[c + 1][:, s]))
                mm_group(pyt[:], pairs)
                nc.vector.tensor_tensor(nxt[c][:, s], pyt[:], nb[:, s],
                                        op=OP.mult)
            nc.vector.copy_predicated(nxt[c][:], Mk[:], cur[c][:])
        cur, nxt = nxt, cur

    # ================= Gaussian ==========================================
    GA = 18
    gs = [sb.tile([P, W + 2 * GA], DT.float32r, name=f"gs{c}", tag=f"lab{c}")
          for c in range(CH)]
    hg = [sb.tile([P, W], DT.float32r, name=f"Emin{c}", tag=f"Emin{c}") for c in range(CH)]
    yo = [sb.tile([P, W], DT.float32, name=f"Emax{c}", tag=f"Emax{c}") for c in range(CH)]
    for c in range(CH):
        nc.vector.tensor_copy(gs[c][:, GA:GA + W], cur[c][:])
        nc.vector.tensor_copy(gs[c][:, 0:GA],
                              cur[c][:, 0:1].to_broadcast((P, GA)))
        nc.vector.tensor_copy(gs[c][:, GA + W:],
                              cur[c][:, W - 1:W].to_broadcast((P, GA)))
    for c in range(CH):
        for h in range(2):
            ph = psY.tile([P, 512], DT.float32, name="pH", tag="psy")
            for j in range(7):
                off = GA + 6 * (j - 3) + h * 512
                nc.tensor.matmul(ph[:], wt[f'G{j}'][:], gs[c][:, off:off + 512],
                                 start=(j == 0), stop=(j == 6))
            nc.scalar.copy(hg[c][:, h * 512:(h + 1) * 512], ph[:])
    for c in range(CH):
        for h in range(2):
            s = slice(h * 512, (h + 1) * 512)
            pv = psY.tile([P, 512], DT.float32, name="pV", tag="psy")
            srcs = [cc for cc in range(CH) if f'B_{c}_{cc}' in wt]
            for i, cc in enumerate(srcs):
                nc.tensor.matmul(pv[:], wt[f'B_{c}_{cc}'][:], hg[cc][:, s],
                                 start=(i == 0), stop=(i == len(srcs) - 1))
            nc.scalar.copy(yo[c][:, s], pv[:])
    for c in range(CH):
        nc.sync.dma_start(y[c * P:(c + 1) * P, :], yo[c][:])


# ------------------------------------------------------------ host driver ---
_CACHE = {}


def _build_program(K=1):
    """Build the program. K>1 wraps the whole per-image kernel body in a
    hardware For_i loop executing it K times back-to-back (identical
    iterations) — used by the ΔK timing harness in test.py."""
    key = ('nc', K)
    if key in _CACHE:
        return _CACHE[key], _CACHE['wpack']
    consts = build_host_consts()
    wnames = sorted(consts.keys())
    wpack = np.zeros((P, len(wnames) * P), np.float32)
    for i, n in enumerate(wnames):
        wpack[:, i * P:(i + 1) * P] = consts[n]

    nc = bacc.Bacc("TRN2", target_bir_lowering=False, debug=False,
                   num_devices=B)
    x_d = nc.dram_tensor("x", [H, W], DT.float32, kind="ExternalInput").ap()
    p_d = nc.dram_tensor("prediction", [H, W], DT.int32,
                         kind="ExternalInput").ap()
    w_d = nc.dram_tensor("wpack", list(wpack.shape), DT.float32,
                         kind="ExternalInput").ap()
    y_d = nc.dram_tensor("y", [H, W], DT.float32, kind="ExternalOutput").ap()
    with tile.TileContext(nc) as tc:
        with ExitStack() as ctx:
            if K > 1:
                with tc.For_i(0, K, 1):
                    build_kernel(ctx, tc, [y_d], [x_d, p_d, w_d])
            else:
                build_kernel(ctx, tc, [y_d], [x_d, p_d, w_d])
    nc.compile()
    _CACHE[key] = nc
    _CACHE['wpack'] = wpack
    return nc, wpack


def _run(x, prediction, trace=False):
    from concourse.bass_utils import run_bass_kernel_spmd
    nc, wpack = _build_program()
    in_maps = []
    for i in range(B):
        in_maps.append({
            "x": np.ascontiguousarray(x[i, 0]).astype(np.float32),
            "prediction": np.ascontiguousarray(prediction[i, 0]).astype(np.int32),
            "wpack": wpack,
        })
    res = run_bass_kernel_spmd(nc, in_maps, core_ids=list(range(B)),
                               trace=trace)
    if trace:
        print(f"HW exec time: {res.exec_time_ns} ns "
              f"(mean {res.mean_exec_time_ns} ns, "
              f"slowest core {res.max_exec_time_core_id})")
        if res.instructions_and_trace:
            print("trace:", res.instructions_and_trace[1])
    out = np.stack([res.results[i]["y"] for i in range(B)], axis=0)
    return out[:, None, :, :].astype(np.float32)


def kernel(x: np.ndarray, prediction: np.ndarray) -> np.ndarray:
    return _run(x, prediction, trace=False)


def kernel_traced(x, prediction, trace=True):
    return _run(x, prediction, trace=trace)


if __name__ == "__main__":
    xs = np.random.randn(B, 1, H, W).astype(np.float32)
    ps = np.random.randint(0, 19, size=(B, 1, H, W)).astype(np.int32)
    print(kernel(xs, ps).shape)



# revision 18
# speedup vs baseline: 387.8489x; 1.1408x over previous
"""Trainium2 Bass kernel for nn_BoundarySuppressionWithSmoothing.

Full inputs: x [8,1,512,1024] f32, prediction [8,1,512,1024] int32.
Sharding: pure data parallel, image i -> core i.

Per-core algorithm (image I [512,1024], layout A: 4 row-chunks of [128,1024]):
  - boundary detection via exp-encoded morphology on PE + ACT (exp/ln-free
    product compare), masks m3..m0 via a mask-carried dilation chain
  - 4 iterations of masked 3x3 box average with replication padding
  - separable dilated 7x7 Gaussian (dilation 6) via PE banded matmuls
"""
import math
import sys
from contextlib import ExitStack

import numpy as np

sys.path.insert(0, '/opt/trn_rl_repo')

import concourse.bass as bass  # noqa: E402
import concourse.bacc as bacc  # noqa: E402
import concourse.tile as tile  # noqa: E402
from concourse import mybir  # noqa: E402

P = 128
W = 1024
H = 512
CH = 4          # row chunks
B = 8           # batch == cores
ALPHA = 4.6     # morphology exp-encoding scale
PTHR = float(np.exp(4.2))   # product threshold for boundary test
CSEL = 65536.0  # keep-selector weight: out=(Y+C*xm+eps*cur)/(n+C*m+eps)
EPS = 2.0 ** -20
DT = mybir.dt
AF = mybir.ActivationFunctionType
OP = mybir.AluOpType


# ---------------------------------------------------------------- weights ---
def _gauss1d():
    size, sigma = 7, 1.0
    u = np.exp(-((np.arange(size) - 3.0) ** 2) / (2 * sigma ** 2))
    # 2D reference kernel is outer(u,u)/sum => separable 1D = u/sum(u)
    return (u / u.sum()).astype(np.float64)


def _round_fp32r(a):
    """Round fp32 array to fp32r (11 explicit mantissa bits) on host."""
    u = a.astype(np.float32).view(np.uint32).astype(np.uint64)
    u = (u + 0x800) & 0xFFFFF000
    return (u & 0xFFFFFFFF).astype(np.uint32).view(np.float32)


def build_host_consts():
    """All constant weight matrices, as one dict of fp32 arrays [128,x]."""
    c = {}
    tri = np.zeros((P, P), np.float32)
    for k in range(P):
        for d in (-1, 0, 1):
            if 0 <= k + d < P:
                tri[k, k + d] = 1.0   # lhsT[k,m]: out m from in k, |k-m|<=1
    c['T_mid'] = tri
    t_top = tri.copy(); t_top[0, 0] = 2.0
    c['T_top'] = t_top
    t_bot = tri.copy(); t_bot[P - 1, P - 1] = 2.0
    c['T_bot'] = t_bot
    t_up = np.zeros((P, P), np.float32); t_up[P - 1, 0] = 1.0
    c['T_up'] = t_up
    t_dn = np.zeros((P, P), np.float32); t_dn[0, P - 1] = 1.0
    c['T_dn'] = t_dn
    c['I'] = np.eye(P, dtype=np.float32)
    c['CI'] = CSEL * np.eye(P, dtype=np.float32)   # bf16 (exact)
    c['CR'] = CSEL * np.eye(P, dtype=np.float32)   # f32r
    bvec = np.zeros((P, P), np.float32)
    bvec[:, 0] = -4.0; bvec[0, 0] = -3.0      # bv_top
    bvec[:, 1] = -4.0; bvec[P - 1, 1] = -3.0  # bv_bot
    c['BVEC'] = bvec

    g = _gauss1d()
    for j in range(7):
        c[f'G{j}'] = _round_fp32r(np.eye(P, dtype=np.float32) * g[j])
    # vertical gaussian: Wv[R,S] = sum_j g[j] [clamp(R+6(j-3),0,H-1)==S]
    Wv = np.zeros((H, H), np.float64)
    for R in range(H):
        for j in range(7):
            S = min(max(R + 6 * (j - 3), 0), H - 1)
            Wv[R, S] += g[j]
    for c_dst in range(CH):
        for c_src in range(CH):
            if abs(c_dst - c_src) > 1:
                continue
            blk = Wv[c_dst * P:(c_dst + 1) * P, c_src * P:(c_src + 1) * P]
            if not blk.any():
                continue
            # lhsT[k,m] = Wv[dst=128c+m, src=128c'+k]
            c[f'B_{c_dst}_{c_src}'] = _round_fp32r(
                np.ascontiguousarray(blk.T).astype(np.float32))
    return c


# ----------------------------------------------------------------- kernel ---
def build_kernel(ctx: ExitStack, tc: "tile.TileContext", outs, ins):
    nc = tc.nc
    y = outs[0]                       # [512,1024] f32 DRAM
    x, pred, wpack = ins              # wpack [128, NW*128] f32 DRAM

    consts = build_host_consts()
    wnames = sorted(consts.keys())

    sb = ctx.enter_context(tc.tile_pool(name="sb", bufs=1))
    sbR = ctx.enter_context(tc.tile_pool(name="sbR", bufs=2))
    wpool = ctx.enter_context(tc.tile_pool(name="wp", bufs=1))
    psB = ctx.enter_context(tc.tile_pool(name="psB", bufs=2, space="PSUM"))
    psY = ctx.enter_context(tc.tile_pool(name="psY", bufs=4, space="PSUM"))

    # ---- load + prepare weights ----
    wstage = sb.tile([P, len(wnames) * P], DT.float32, tag="wstage")
    nc.sync.dma_start(wstage[:], wpack[:, :len(wnames) * P])
    wt = {}
    BF16_W = {'T_mid', 'T_top', 'T_bot', 'T_up', 'T_dn', 'I', 'CI'}
    for i, name in enumerate(wnames):
        if name == 'BVEC':
            continue
        src = wstage[:, i * P:(i + 1) * P]
        dt_w = DT.bfloat16 if name in BF16_W else DT.float32r
        t = wpool.tile([P, P], dt_w, name=f"w_{name}", tag=f"w_{name}")
        nc.vector.tensor_copy(t[:], src)
        wt[name] = t
    # fp32r variants of vertical matrices for the value path
    for name in ('T_mid', 'T_top', 'T_bot', 'T_up', 'T_dn'):
        t = wpool.tile([P, P], DT.float32r, name=f"wr_{name}", tag=f"wr_{name}")
        i = wnames.index(name)
        nc.vector.tensor_copy(t[:], wstage[:, i * P:(i + 1) * P])
        wt['R' + name[1:]] = t

    def TRv(c):
        return wt['T_top'] if c == 0 else (wt['T_bot'] if c == CH - 1 else wt['T_mid'])

    def Rv(c):
        return wt['R_top'] if c == 0 else (wt['R_bot'] if c == CH - 1 else wt['R_mid'])

    # ---- const bias vectors ----
    def make_const(val, tag):
        t = sb.tile([P, 1], DT.float32, tag=tag)
        nc.vector.memset(t[:], val)
        return t

    b_enc_max = make_const(-9.0 * ALPHA, "b_enc_max")
    b_enc_min = make_const(+9.0 * ALPHA, "b_enc_min")
    b_eps = make_const(EPS, "b_eps")
    bv_mid = make_const(-4.0, "bv_mid")
    ib = wnames.index('BVEC')
    bv_top = sb.tile([P, 1], DT.float32, name="bv_top", tag="bv_top")
    nc.vector.tensor_copy(bv_top[:], wstage[:, ib * P:ib * P + 1])
    bv_bot = sb.tile([P, 1], DT.float32, name="bv_bot", tag="bv_bot")
    nc.vector.tensor_copy(bv_bot[:], wstage[:, ib * P + 1:ib * P + 2])
    one_c = make_const(1.0, "one_c")

    def bv(c):
        return bv_top if c == 0 else (bv_bot if c == CH - 1 else bv_mid)

    # ---- persistent image buffers ----
    lab = [sb.tile([P, W], DT.int32, name=f"lab{c}", tag=f"lab{c}") for c in range(CH)]
    OA = [sb.tile([P, W], DT.float32, name=f"OA{c}", tag=f"OA{c}") for c in range(CH)]
    OB = [sb.tile([P, W], DT.float32, name=f"OB{c}", tag=f"OB{c}") for c in range(CH)]
    # labels first: the boundary phase (ACT encodes -> PE) is the kernel's
    # critical-path head; x is not needed until the U loop.
    for c in range(CH):
        nc.sync.dma_start(lab[c][:], pred[c * P:(c + 1) * P, :])
    for c in range(CH):
        nc.sync.dma_start(OA[c][:], x[c * P:(c + 1) * P, :])

    GW = W + 2

    def gtile(tag, dtype, guard_val, pool=sb):
        ts = [pool.tile([P, GW], dtype, name=f"{tag}{c}", tag=f"{tag}{c}") for c in range(CH)]
        for c in range(CH):
            for ap in (ts[c][:, 0:1], ts[c][:, GW - 1:GW]):
                if dtype == DT.float32r:
                    ap = ap.bitcast(DT.float32)
                nc.vector.memset(ap, guard_val)
        return ts

    Emax = gtile("Emax", DT.bfloat16, 0.0)
    Emin = gtile("Emin", DT.bfloat16, 0.0)
    m = [gtile(f"m{i}_", DT.bfloat16, 1.0) for i in range(4)]
    xm = gtile("xm", DT.float32r, 0.0)
    HN = [sb.tile([P, W], DT.bfloat16, name=f"HN{c}", tag=f"HMa{c}") for c in range(CH)]
    HMa = [sb.tile([P, W], DT.bfloat16, name=f"HMa{c}", tag=f"HMa{c}") for c in range(CH)]
    hlr = [sb.tile([P, W], DT.float32r, name=f"hlr{c}", tag=f"hlr{c}") for c in range(CH)]

    def data(t):
        return t[:, 1:W + 1]

    def shl(t):
        return t[:, 0:W]

    def shr(t):
        return t[:, 2:W + 2]

    def mm_group(pt, pairs, preloaded=False):
        # split into N=512 sub-matmuls (PSUM bank limit); weight-major order
        # so consecutive matmuls share the stationary operand (fewer LDW).
        # preloaded=True: PSUM already initialized (e.g. by an ACT write);
        # accumulate onto it instead of resetting.
        n = pt.shape[1]
        halves = list(range(0, n, 512))
        for i, (lhsT, rhs) in enumerate(pairs):
            for h0 in halves:
                nc.tensor.matmul(pt[:, h0:h0 + 512], lhsT,
                                 rhs[:, h0:h0 + 512],
                                 start=(False if preloaded else (i == 0)),
                                 stop=(i == len(pairs) - 1),
                                 skip_group_check=preloaded)

    # ================= Phase M: encode + boundary masks ===================
    for c in range(CH):
        nc.scalar.activation(data(Emax[c]), lab[c][:], AF.Exp,
                             bias=b_enc_max[:], scale=ALPHA)
        nc.scalar.activation(data(Emin[c]), lab[c][:], AF.Exp,
                             bias=b_enc_min[:], scale=-ALPHA)
    for c in range(CH):
        nc.vector.tensor_tensor(HN[c][:], shl(Emin[c]), shr(Emin[c]), op=OP.add)
        nc.vector.tensor_tensor(HN[c][:], HN[c][:], data(Emin[c]), op=OP.add)
    for c in range(CH):
        p1 = psB.tile([P, W], DT.float32, name="pS1", tag="psb")
        pairs = [(wt['T_mid'][:], data(Emax[c])),
                 (wt['I'][:], shl(Emax[c])),
                 (wt['I'][:], shr(Emax[c]))]
        if c > 0:
            pairs.append((wt['T_up'][:], data(Emax[c - 1])))
        if c < CH - 1:
            pairs.append((wt['T_dn'][:], data(Emax[c + 1])))
        mm_group(p1[:], pairs)
        sc1 = sbR.tile([P, W], DT.bfloat16, name="sc1", tag="nb")
        nc.scalar.copy(sc1[:], p1[:])

        p2 = psB.tile([P, W], DT.float32, name="pS2", tag="psb")
        pairs = [(wt['T_mid'][:], HN[c][:])]
        if c > 0:
            pairs.append((wt['T_up'][:], HN[c - 1][:]))
        if c < CH - 1:
            pairs.append((wt['T_dn'][:], HN[c + 1][:]))
        mm_group(p2[:], pairs)
        pb = sbR.tile([P, W], DT.bfloat16, name="pb", tag="zt")
        nc.vector.tensor_tensor(pb[:], sc1[:], p2[:], op=OP.mult)
        nc.vector.tensor_scalar(data(m[3][c]), pb[:], PTHR, None, op0=OP.is_lt)

    # ================= Chain: m3 -> m2 -> m1 -> m0 ========================
    for k in range(3):
        mp, mn = m[3 - k], m[2 - k]
        for c in range(CH):
            ps = psB.tile([P, W], DT.float32, name="pCh", tag="psb")
            pairs = [(wt['T_mid'][:], data(mp[c])),
                     (wt['I'][:], shl(mp[c])),
                     (wt['I'][:], shr(mp[c]))]
            if c > 0:
                pairs.append((wt['T_up'][:], data(mp[c - 1])))
            if c < CH - 1:
                pairs.append((wt['T_dn'][:], data(mp[c + 1])))
            mm_group(ps[:], pairs)
            nc.scalar.activation(data(mn[c]), ps[:], AF.Relu, bias=bv(c)[:],
                                 scale=1.0)

    # ================= U loop =============================================
    # Output stage uses a big-constant selector folded into the matmuls:
    #   num = Y + C*xm + eps*cur   (Y = box3(x*m), xm = cur*m)
    #   den = n + C*m + eps        (n = box3(m))
    #   out = num / den
    # m=1 -> out ~= cur (err ~(|Y|+n|cur|)/C); m=0,n>0 -> out ~= Y/n;
    # m=0,n=0 -> out = eps*cur/eps = cur.  No reciprocal / predication.
    cur, nxt = OA, OB
    for it in range(4):
        mi = m[it]
        for c in range(CH):
            nc.gpsimd.tensor_tensor(xm[c][:, 1:W + 1], cur[c][:], data(mi[c]),
                                    op=OP.mult)
            nc.vector.tensor_tensor(HMa[c][:], shl(mi[c]), shr(mi[c]), op=OP.add)
        for c in range(CH):
            # HMa := full hsum3_rep(m) = mL + mR + m, with edge fixes
            nc.vector.tensor_tensor(HMa[c][:], HMa[c][:], data(mi[c]), op=OP.add)
            nc.vector.tensor_scalar(HMa[c][:, 0:1], mi[c][:, 1:2], 2.0, None,
                                    op0=OP.mult)
            nc.vector.tensor_tensor(HMa[c][:, 0:1], HMa[c][:, 0:1],
                                    mi[c][:, 2:3], op=OP.add)
            nc.vector.tensor_scalar(HMa[c][:, W - 1:W], mi[c][:, W:W + 1], 2.0,
                                    None, op0=OP.mult)
            nc.vector.tensor_tensor(HMa[c][:, W - 1:W], HMa[c][:, W - 1:W],
                                    mi[c][:, W - 1:W], op=OP.add)
            # hlr := xmL + xmR (DVE), edge fixes, then SH := hlr + xm (gpsimd)
            nc.vector.tensor_tensor(hlr[c][:], shl(xm[c]), shr(xm[c]), op=OP.add)
            nc.vector.tensor_tensor(hlr[c][:, 0:1], hlr[c][:, 0:1],
                                    xm[c][:, 1:2], op=OP.add)
            nc.vector.tensor_tensor(hlr[c][:, W - 1:W], hlr[c][:, W - 1:W],
                                    xm[c][:, W:W + 1], op=OP.add)
        for c in range(CH):
            nc.gpsimd.tensor_tensor(hlr[c][:], hlr[c][:], xm[c][:, 1:W + 1],
                                    op=OP.add)
        for c in range(CH):
            pn = psB.tile([P, W], DT.float32, name="pN", tag="psb")
            pairs = [(TRv(c)[:], HMa[c][:]),
                     (wt['CI'][:], data(mi[c]))]
            if c > 0:
                pairs.append((wt['T_up'][:], HMa[c - 1][:]))
            if c < CH - 1:
                pairs.append((wt['T_dn'][:], HMa[c + 1][:]))
            mm_group(pn[:], pairs)
            ns = sbR.tile([P, W], DT.float32, name="ns", tag="nb")
            nc.scalar.activation(ns[:], pn[:], AF.Copy, bias=EPS, scale=1.0)
            nc.vector.reciprocal(ns[:], ns[:])
            Ys = sbR.tile([P, W], DT.float32, name="Ys", tag="Ys")
            for h in range(2):
                s = slice(h * 512, (h + 1) * 512)
                sg = slice(1 + h * 512, 1 + (h + 1) * 512)
                pyt = psY.tile([P, 512], DT.float32, name="pY", tag="psy")
                # preload PSUM with eps*cur (the m=0,n=0 keep term), then
                # accumulate Y + C*xm on top.
                nc.scalar.activation(pyt[:], cur[c][:, s], AF.Copy, bias=0.0,
                                     scale=EPS)
                pairs = [(Rv(c)[:], hlr[c][:, s]),
                         (wt['CR'][:], xm[c][:, sg])]
                if c > 0:
                    pairs.append((wt['R_up'][:], hlr[c - 1][:, s]))
                if c < CH - 1:
                    pairs.append((wt['R_dn'][:], hlr[c + 1][:, s]))
                mm_group(pyt[:], pairs, preloaded=True)
                nc.scalar.copy(Ys[:, s], pyt[:])
            nc.vector.tensor_tensor(nxt[c][:], Ys[:], ns[:], op=OP.mult)
        cur, nxt = nxt, cur

    # ================= Gaussian ==========================================
    GA = 18
    gs = [sb.tile([P, W + 2 * GA], DT.float32r, name=f"gs{c}", tag=f"lab{c}")
          for c in range(CH)]
    hg = [sb.tile([P, W], DT.float32r, name=f"Emin{c}", tag=f"Emin{c}") for c in range(CH)]
    yo = [sb.tile([P, W], DT.float32, name=f"Emax{c}", tag=f"Emax{c}") for c in range(CH)]
    for c in range(CH):
        nc.vector.tensor_copy(gs[c][:, GA:GA + W], cur[c][:])
        nc.vector.tensor_copy(gs[c][:, 0:GA],
                              cur[c][:, 0:1].to_broadcast((P, GA)))
        nc.vector.tensor_copy(gs[c][:, GA + W:],
                              cur[c][:, W - 1:W].to_broadcast((P, GA)))
    for c in range(CH):
        for h in range(2):
            ph = psY.tile([P, 512], DT.float32, name="pH", tag="psy")
            for j in range(7):
                off = GA + 6 * (j - 3) + h * 512
                nc.tensor.matmul(ph[:], wt[f'G{j}'][:], gs[c][:, off:off + 512],
                                 start=(j == 0), stop=(j == 6))
            nc.scalar.copy(hg[c][:, h * 512:(h + 1) * 512], ph[:])
    for c in range(CH):
        for h in range(2):
            s = slice(h * 512, (h + 1) * 512)
            pv = psY.tile([P, 512], DT.float32, name="pV", tag="psy")
            srcs = [cc for cc in range(CH) if f'B_{c}_{cc}' in wt]
            for i, cc in enumerate(srcs):
                nc.tensor.matmul(pv[:], wt[f'B_{c}_{cc}'][:], hg[cc][:, s],
                                 start=(i == 0), stop=(i == len(srcs) - 1))
            nc.scalar.copy(yo[c][:, s], pv[:])
    for c in range(CH):
        nc.sync.dma_start(y[c * P:(c + 1) * P, :], yo[c][:])


# ------------------------------------------------------------ host driver ---
_CACHE = {}


def _build_program(K=1):
    """Build the program. K>1 wraps the whole per-image kernel body in a
    hardware For_i loop executing it K times back-to-back (identical
    iterations) — used by the ΔK timing harness in test.py."""
    key = ('nc', K)
    if key in _CACHE:
        return _CACHE[key], _CACHE['wpack']
    consts = build_host_consts()
    wnames = sorted(consts.keys())
    wpack = np.zeros((P, len(wnames) * P), np.float32)
    for i, n in enumerate(wnames):
        wpack[:, i * P:(i + 1) * P] = consts[n]

    nc = bacc.Bacc("TRN2", target_bir_lowering=False, debug=False,
                   num_devices=B)
    x_d = nc.dram_tensor("x", [H, W], DT.float32, kind="ExternalInput").ap()
    p_d = nc.dram_tensor("prediction", [H, W], DT.int32,
                         kind="ExternalInput").ap()
    w_d = nc.dram_tensor("wpack", list(wpack.shape), DT.float32,
                         kind="ExternalInput").ap()
    y_d = nc.dram_tensor("y", [H, W], DT.float32, kind="ExternalOutput").ap()
    with tile.TileContext(nc) as tc:
        with ExitStack() as ctx:
            if K > 1:
                with tc.For_i(0, K, 1):
                    build_kernel(ctx, tc, [y_d], [x_d, p_d, w_d])
            else:
                build_kernel(ctx, tc, [y_d], [x_d, p_d, w_d])
    nc.compile()
    _CACHE[key] = nc
    _CACHE['wpack'] = wpack
    return nc, wpack


def _run(x, prediction, trace=False):
    from concourse.bass_utils import run_bass_kernel_spmd
    nc, wpack = _build_program()
    in_maps = []
    for i in range(B):
        in_maps.append({
            "x": np.ascontiguousarray(x[i, 0]).astype(np.float32),
            "prediction": np.ascontiguousarray(prediction[i, 0]).astype(np.int32),
            "wpack": wpack,
        })
    res = run_bass_kernel_spmd(nc, in_maps, core_ids=list(range(B)),
                               trace=trace)
    if trace:
        print(f"HW exec time: {res.exec_time_ns} ns "
              f"(mean {res.mean_exec_time_ns} ns, "
              f"slowest core {res.max_exec_time_core_id})")
        if res.instructions_and_trace:
            print("trace:", res.instructions_and_trace[1])
    out = np.stack([res.results[i]["y"] for i in range(B)], axis=0)
    return out[:, None, :, :].astype(np.float32)


def kernel(x: np.ndarray, prediction: np.ndarray) -> np.ndarray:
    return _run(x, prediction, trace=False)


def kernel_traced(x, prediction, trace=True):
    return _run(x, prediction, trace=trace)


if __name__ == "__main__":
    xs = np.random.randn(B, 1, H, W).astype(np.float32)
    ps = np.random.randint(0, 19, size=(B, 1, H, W)).astype(np.int32)
    print(kernel(xs, ps).shape)



# revision 24
# speedup vs baseline: 395.6302x; 1.0201x over previous
"""Trainium2 Bass kernel for nn_BoundarySuppressionWithSmoothing.

Full inputs: x [8,1,512,1024] f32, prediction [8,1,512,1024] int32.
Sharding: pure data parallel, image i -> core i.

Per-core algorithm (image I [512,1024], layout A: 4 row-chunks of [128,1024]):
  - boundary detection via exp-encoded morphology on PE + ACT (exp/ln-free
    product compare), masks m3..m0 via a mask-carried dilation chain
  - 4 iterations of masked 3x3 box average with replication padding
  - separable dilated 7x7 Gaussian (dilation 6) via PE banded matmuls
"""
import math
import sys
from contextlib import ExitStack

import numpy as np

sys.path.insert(0, '/opt/trn_rl_repo')

import concourse.bass as bass  # noqa: E402
import concourse.bacc as bacc  # noqa: E402
import concourse.tile as tile  # noqa: E402
from concourse import mybir  # noqa: E402

P = 128
W = 1024
H = 512
CH = 4          # row chunks
B = 8           # batch == cores
ALPHA = 4.6     # morphology exp-encoding scale
PTHR = float(np.exp(4.2))   # product threshold for boundary test
CSEL = 65536.0  # keep-selector weight: out=(Y+C*xm+eps*cur)/(n+C*m+eps)
EPS = 2.0 ** -20
DT = mybir.dt
AF = mybir.ActivationFunctionType
OP = mybir.AluOpType


# ---------------------------------------------------------------- weights ---
def _gauss1d():
    size, sigma = 7, 1.0
    u = np.exp(-((np.arange(size) - 3.0) ** 2) / (2 * sigma ** 2))
    # 2D reference kernel is outer(u,u)/sum => separable 1D = u/sum(u)
    return (u / u.sum()).astype(np.float64)


def _round_fp32r(a):
    """Round fp32 array to fp32r (11 explicit mantissa bits) on host."""
    u = a.astype(np.float32).view(np.uint32).astype(np.uint64)
    u = (u + 0x800) & 0xFFFFF000
    return (u & 0xFFFFFFFF).astype(np.uint32).view(np.float32)


def build_host_consts():
    """All constant weight matrices, as one dict of fp32 arrays [128,x]."""
    c = {}
    tri = np.zeros((P, P), np.float32)
    for k in range(P):
        for d in (-1, 0, 1):
            if 0 <= k + d < P:
                tri[k, k + d] = 1.0   # lhsT[k,m]: out m from in k, |k-m|<=1
    c['T_mid'] = tri
    t_top = tri.copy(); t_top[0, 0] = 2.0
    c['T_top'] = t_top
    t_bot = tri.copy(); t_bot[P - 1, P - 1] = 2.0
    c['T_bot'] = t_bot
    t_up = np.zeros((P, P), np.float32); t_up[P - 1, 0] = 1.0
    c['T_up'] = t_up
    t_dn = np.zeros((P, P), np.float32); t_dn[0, P - 1] = 1.0
    c['T_dn'] = t_dn
    c['I'] = np.eye(P, dtype=np.float32)
    c['CI'] = CSEL * np.eye(P, dtype=np.float32)   # bf16 (exact)
    c['CR'] = CSEL * np.eye(P, dtype=np.float32)   # f32r
    bvec = np.zeros((P, P), np.float32)
    bvec[:, 0] = -4.0; bvec[0, 0] = -3.0      # bv_top
    bvec[:, 1] = -4.0; bvec[P - 1, 1] = -3.0  # bv_bot
    c['BVEC'] = bvec

    g = _gauss1d()
    for j in range(7):
        c[f'G{j}'] = _round_fp32r(np.eye(P, dtype=np.float32) * g[j])
    # vertical gaussian: Wv[R,S] = sum_j g[j] [clamp(R+6(j-3),0,H-1)==S]
    Wv = np.zeros((H, H), np.float64)
    for R in range(H):
        for j in range(7):
            S = min(max(R + 6 * (j - 3), 0), H - 1)
            Wv[R, S] += g[j]
    for c_dst in range(CH):
        for c_src in range(CH):
            if abs(c_dst - c_src) > 1:
                continue
            blk = Wv[c_dst * P:(c_dst + 1) * P, c_src * P:(c_src + 1) * P]
            if not blk.any():
                continue
            # lhsT[k,m] = Wv[dst=128c+m, src=128c'+k]
            c[f'B_{c_dst}_{c_src}'] = _round_fp32r(
                np.ascontiguousarray(blk.T).astype(np.float32))
    return c


# ----------------------------------------------------------------- kernel ---
def build_kernel(ctx: ExitStack, tc: "tile.TileContext", outs, ins):
    nc = tc.nc
    y = outs[0]                       # [512,1024] f32 DRAM
    x, pred, wpack = ins              # wpack [128, NW*128] f32 DRAM

    consts = build_host_consts()
    wnames = sorted(consts.keys())

    sb = ctx.enter_context(tc.tile_pool(name="sb", bufs=1))
    sbR = ctx.enter_context(tc.tile_pool(name="sbR", bufs=2))
    wpool = ctx.enter_context(tc.tile_pool(name="wp", bufs=1))
    psB = ctx.enter_context(tc.tile_pool(name="psB", bufs=2, space="PSUM"))
    psY = ctx.enter_context(tc.tile_pool(name="psY", bufs=4, space="PSUM"))

    # ---- load + prepare weights ----
    wstage = sb.tile([P, len(wnames) * P], DT.float32, tag="wstage")
    nc.sync.dma_start(wstage[:], wpack[:, :len(wnames) * P])
    wt = {}
    BF16_W = {'T_mid', 'T_top', 'T_bot', 'T_up', 'T_dn', 'I', 'CI'}
    for i, name in enumerate(wnames):
        if name == 'BVEC':
            continue
        src = wstage[:, i * P:(i + 1) * P]
        dt_w = DT.bfloat16 if name in BF16_W else DT.float32r
        t = wpool.tile([P, P], dt_w, name=f"w_{name}", tag=f"w_{name}")
        nc.vector.tensor_copy(t[:], src)
        wt[name] = t
    # fp32r variants of vertical matrices for the value path
    for name in ('T_mid', 'T_top', 'T_bot', 'T_up', 'T_dn'):
        t = wpool.tile([P, P], DT.float32r, name=f"wr_{name}", tag=f"wr_{name}")
        i = wnames.index(name)
        nc.vector.tensor_copy(t[:], wstage[:, i * P:(i + 1) * P])
        wt['R' + name[1:]] = t

    def TRv(c):
        return wt['T_top'] if c == 0 else (wt['T_bot'] if c == CH - 1 else wt['T_mid'])

    def Rv(c):
        return wt['R_top'] if c == 0 else (wt['R_bot'] if c == CH - 1 else wt['R_mid'])

    # ---- const bias vectors ----
    def make_const(val, tag):
        t = sb.tile([P, 1], DT.float32, tag=tag)
        nc.vector.memset(t[:], val)
        return t

    b_enc_max = make_const(-9.0 * ALPHA, "b_enc_max")
    b_enc_min = make_const(+9.0 * ALPHA, "b_enc_min")
    b_eps = make_const(EPS, "b_eps")
    bv_mid = make_const(-4.0, "bv_mid")
    ib = wnames.index('BVEC')
    bv_top = sb.tile([P, 1], DT.float32, name="bv_top", tag="bv_top")
    nc.vector.tensor_copy(bv_top[:], wstage[:, ib * P:ib * P + 1])
    bv_bot = sb.tile([P, 1], DT.float32, name="bv_bot", tag="bv_bot")
    nc.vector.tensor_copy(bv_bot[:], wstage[:, ib * P + 1:ib * P + 2])
    one_c = make_const(1.0, "one_c")

    def bv(c):
        return bv_top if c == 0 else (bv_bot if c == CH - 1 else bv_mid)

    # ---- persistent image buffers ----
    lab = [sb.tile([P, W], DT.int32, name=f"lab{c}", tag=f"lab{c}") for c in range(CH)]
    OA = [sb.tile([P, W], DT.float32, name=f"OA{c}", tag=f"OA{c}") for c in range(CH)]
    OB = [sb.tile([P, W], DT.float32, name=f"OB{c}", tag=f"OB{c}") for c in range(CH)]
    # labels first: the boundary phase (ACT encodes -> PE) is the kernel's
    # critical-path head; x is not needed until the U loop.
    for c in range(CH):
        nc.sync.dma_start(lab[c][:], pred[c * P:(c + 1) * P, :])
    for c in range(CH):
        nc.sync.dma_start(OA[c][:], x[c * P:(c + 1) * P, :])

    GW = W + 2

    def gtile(tag, dtype, guard_val, pool=sb):
        ts = [pool.tile([P, GW], dtype, name=f"{tag}{c}", tag=f"{tag}{c}") for c in range(CH)]
        for c in range(CH):
            for ap in (ts[c][:, 0:1], ts[c][:, GW - 1:GW]):
                if dtype == DT.float32r:
                    ap = ap.bitcast(DT.float32)
                nc.vector.memset(ap, guard_val)
        return ts

    Emax = gtile("Emax", DT.bfloat16, 0.0)
    Emin = gtile("Emin", DT.bfloat16, 0.0)
    m = [gtile(f"m{i}_", DT.bfloat16, 1.0) for i in range(4)]
    xm = gtile("xm", DT.float32r, 0.0)
    HN = [sb.tile([P, W], DT.bfloat16, name=f"HN{c}", tag=f"HMa{c}") for c in range(CH)]
    HMa = [sb.tile([P, W], DT.bfloat16, name=f"HMa{c}", tag=f"HMa{c}") for c in range(CH)]
    hlr = [sb.tile([P, W], DT.float32r, name=f"hlr{c}", tag=f"hlr{c}") for c in range(CH)]

    def data(t):
        return t[:, 1:W + 1]

    def grd(t):
        # replicate edge cols into the guards: hsum3 over data cols then
        # matches replication padding exactly (and is equivalent to the
        # geodesic border for the erosion chain, where phantom==center).
        nc.vector.tensor_copy(t[:, 0:1], t[:, 1:2])
        nc.vector.tensor_copy(t[:, W + 1:W + 2], t[:, W:W + 1])

    def shl(t):
        return t[:, 0:W]

    def shr(t):
        return t[:, 2:W + 2]

    def mm_group(pt, pairs, preloaded=False):
        # split into N=512 sub-matmuls (PSUM bank limit); weight-major order
        # so consecutive matmuls share the stationary operand (fewer LDW).
        # preloaded=True: PSUM already initialized (e.g. by an ACT write);
        # accumulate onto it instead of resetting.
        n = pt.shape[1]
        halves = list(range(0, n, 512))
        for i, (lhsT, rhs) in enumerate(pairs):
            for h0 in halves:
                nc.tensor.matmul(pt[:, h0:h0 + 512], lhsT,
                                 rhs[:, h0:h0 + 512],
                                 start=(False if preloaded else (i == 0)),
                                 stop=(i == len(pairs) - 1),
                                 skip_group_check=preloaded)

    # ================= Phase M: encode + boundary masks ===================
    for c in range(CH):
        nc.scalar.activation(data(Emax[c]), lab[c][:], AF.Exp,
                             bias=b_enc_max[:], scale=ALPHA)
        nc.scalar.activation(data(Emin[c]), lab[c][:], AF.Exp,
                             bias=b_enc_min[:], scale=-ALPHA)
    for c in range(CH):
        nc.vector.tensor_tensor(HN[c][:], shl(Emin[c]), shr(Emin[c]), op=OP.add)
        nc.vector.tensor_tensor(HN[c][:], HN[c][:], data(Emin[c]), op=OP.add)
    for c in range(CH):
        p1 = psB.tile([P, W], DT.float32, name="pS1", tag="psb")
        hxe = sbR.tile([P, W], DT.bfloat16, name="hxe", tag="hx")
        nc.vector.tensor_tensor(hxe[:], shl(Emax[c]), shr(Emax[c]), op=OP.add)
        pairs = [(wt['T_mid'][:], data(Emax[c])),
                 (wt['I'][:], hxe[:])]
        if c > 0:
            pairs.append((wt['T_up'][:], data(Emax[c - 1])))
        if c < CH - 1:
            pairs.append((wt['T_dn'][:], data(Emax[c + 1])))
        mm_group(p1[:], pairs)
        sc1 = sbR.tile([P, W], DT.bfloat16, name="sc1", tag="nb")
        nc.scalar.copy(sc1[:], p1[:])

        p2 = psB.tile([P, W], DT.float32, name="pS2", tag="psb")
        pairs = [(wt['T_mid'][:], HN[c][:])]
        if c > 0:
            pairs.append((wt['T_up'][:], HN[c - 1][:]))
        if c < CH - 1:
            pairs.append((wt['T_dn'][:], HN[c + 1][:]))
        mm_group(p2[:], pairs)
        pb = sbR.tile([P, W], DT.bfloat16, name="pb", tag="zt")
        nc.vector.tensor_tensor(pb[:], sc1[:], p2[:], op=OP.mult)
        nc.vector.tensor_scalar(data(m[3][c]), pb[:], PTHR, None, op0=OP.is_lt)
        grd(m[3][c])

    # ================= Chain: m3 -> m2 -> m1 -> m0 ========================
    for k in range(3):
        mp, mn = m[3 - k], m[2 - k]
        for c in range(CH):
            ps = psB.tile([P, W], DT.float32, name="pCh", tag="psb")
            hx = sbR.tile([P, W], DT.bfloat16, name="hx", tag="hx")
            nc.vector.tensor_tensor(hx[:], shl(mp[c]), shr(mp[c]), op=OP.add)
            pairs = [(wt['T_mid'][:], data(mp[c])),
                     (wt['I'][:], hx[:])]
            if c > 0:
                pairs.append((wt['T_up'][:], data(mp[c - 1])))
            if c < CH - 1:
                pairs.append((wt['T_dn'][:], data(mp[c + 1])))
            mm_group(ps[:], pairs)
            nc.scalar.activation(data(mn[c]), ps[:], AF.Relu, bias=bv(c)[:],
                                 scale=1.0)
            grd(mn[c])

    # ================= U loop =============================================
    # Output stage uses a big-constant selector folded into the matmuls:
    #   num = Y + C*xm + eps*cur   (Y = box3(x*m), xm = cur*m)
    #   den = n + C*m + eps        (n = box3(m))
    #   out = num / den
    # m=1 -> out ~= cur (err ~(|Y|+n|cur|)/C); m=0,n>0 -> out ~= Y/n;
    # m=0,n=0 -> out = eps*cur/eps = cur.  No reciprocal / predication.
    cur, nxt = OA, OB
    for it in range(4):
        mi = m[it]
        for c in range(CH):
            nc.gpsimd.tensor_tensor(xm[c][:, 1:W + 1], cur[c][:], data(mi[c]),
                                    op=OP.mult)
            nc.vector.tensor_tensor(HMa[c][:], shl(mi[c]), shr(mi[c]), op=OP.add)
        for c in range(CH):
            # HMa := full hsum3_rep(m) = mL + mR + m, with edge fixes
            nc.vector.tensor_tensor(HMa[c][:], HMa[c][:], data(mi[c]), op=OP.add)
            nc.vector.tensor_scalar(HMa[c][:, 0:1], mi[c][:, 1:2], 2.0, None,
                                    op0=OP.mult)
            nc.vector.tensor_tensor(HMa[c][:, 0:1], HMa[c][:, 0:1],
                                    mi[c][:, 2:3], op=OP.add)
            nc.vector.tensor_scalar(HMa[c][:, W - 1:W], mi[c][:, W:W + 1], 2.0,
                                    None, op0=OP.mult)
            nc.vector.tensor_tensor(HMa[c][:, W - 1:W], HMa[c][:, W - 1:W],
                                    mi[c][:, W - 1:W], op=OP.add)
            # hlr := xmL + xmR (DVE), edge fixes, then SH := hlr + xm (gpsimd)
            nc.vector.tensor_tensor(hlr[c][:], shl(xm[c]), shr(xm[c]), op=OP.add)
            nc.vector.tensor_tensor(hlr[c][:, 0:1], hlr[c][:, 0:1],
                                    xm[c][:, 1:2], op=OP.add)
            nc.vector.tensor_tensor(hlr[c][:, W - 1:W], hlr[c][:, W - 1:W],
                                    xm[c][:, W:W + 1], op=OP.add)
        for c in range(CH):
            nc.gpsimd.tensor_tensor(hlr[c][:], hlr[c][:], xm[c][:, 1:W + 1],
                                    op=OP.add)
        for c in range(CH):
            pn = psB.tile([P, W], DT.float32, name="pN", tag="psb")
            pairs = [(TRv(c)[:], HMa[c][:]),
                     (wt['CI'][:], data(mi[c]))]
            if c > 0:
                pairs.append((wt['T_up'][:], HMa[c - 1][:]))
            if c < CH - 1:
                pairs.append((wt['T_dn'][:], HMa[c + 1][:]))
            mm_group(pn[:], pairs)
            ns = sbR.tile([P, W], DT.float32, name="ns", tag="nb")
            nc.scalar.activation(ns[:], pn[:], AF.Copy, bias=EPS, scale=1.0)
            nc.vector.reciprocal(ns[:], ns[:])
            Ys = sbR.tile([P, W], DT.float32, name="Ys", tag="Ys")
            for h in range(2):
                s = slice(h * 512, (h + 1) * 512)
                sg = slice(1 + h * 512, 1 + (h + 1) * 512)
                pyt = psY.tile([P, 512], DT.float32, name="pY", tag="psy")
                # preload PSUM with eps*cur (the m=0,n=0 keep term), then
                # accumulate Y + C*xm on top.
                nc.scalar.activation(pyt[:], cur[c][:, s], AF.Copy, bias=0.0,
                                     scale=EPS)
                pairs = [(Rv(c)[:], hlr[c][:, s]),
                         (wt['CR'][:], xm[c][:, sg])]
                if c > 0:
                    pairs.append((wt['R_up'][:], hlr[c - 1][:, s]))
                if c < CH - 1:
                    pairs.append((wt['R_dn'][:], hlr[c + 1][:, s]))
                mm_group(pyt[:], pairs, preloaded=True)
                nc.scalar.copy(Ys[:, s], pyt[:])
            nc.vector.tensor_tensor(nxt[c][:], Ys[:], ns[:], op=OP.mult)
        cur, nxt = nxt, cur

    # ================= Gaussian ==========================================
    GA = 18
    gs = [sb.tile([P, W + 2 * GA], DT.float32r, name=f"gs{c}", tag=f"lab{c}")
          for c in range(CH)]
    hg = [sb.tile([P, W], DT.float32r, name=f"Emin{c}", tag=f"Emin{c}") for c in range(CH)]
    yo = [sb.tile([P, W], DT.float32, name=f"Emax{c}", tag=f"Emax{c}") for c in range(CH)]
    for c in range(CH):
        nc.vector.tensor_copy(gs[c][:, GA:GA + W], cur[c][:])
        nc.vector.tensor_copy(gs[c][:, 0:GA],
                              cur[c][:, 0:1].to_broadcast((P, GA)))
        nc.vector.tensor_copy(gs[c][:, GA + W:],
                              cur[c][:, W - 1:W].to_broadcast((P, GA)))
    for c in range(CH):
        for h in range(2):
            ph = psY.tile([P, 512], DT.float32, name="pH", tag="psy")
            for j in range(7):
                off = GA + 6 * (j - 3) + h * 512
                nc.tensor.matmul(ph[:], wt[f'G{j}'][:], gs[c][:, off:off + 512],
                                 start=(j == 0), stop=(j == 6))
            nc.scalar.copy(hg[c][:, h * 512:(h + 1) * 512], ph[:])
    for c in range(CH):
        for h in range(2):
            s = slice(h * 512, (h + 1) * 512)
            pv = psY.tile([P, 512], DT.float32, name="pV", tag="psy")
            srcs = [cc for cc in range(CH) if f'B_{c}_{cc}' in wt]
            for i, cc in enumerate(srcs):
                nc.tensor.matmul(pv[:], wt[f'B_{c}_{cc}'][:], hg[cc][:, s],
                                 start=(i == 0), stop=(i == len(srcs) - 1))
            nc.scalar.copy(yo[c][:, s], pv[:])
    for c in range(CH):
        nc.sync.dma_start(y[c * P:(c + 1) * P, :], yo[c][:])


# ------------------------------------------------------------ host driver ---
_CACHE = {}


def _build_program(K=1):
    """Build the program. K>1 wraps the whole per-image kernel body in a
    hardware For_i loop executing it K times back-to-back (identical
    iterations) — used by the ΔK timing harness in test.py."""
    key = ('nc', K)
    if key in _CACHE:
        return _CACHE[key], _CACHE['wpack']
    consts = build_host_consts()
    wnames = sorted(consts.keys())
    wpack = np.zeros((P, len(wnames) * P), np.float32)
    for i, n in enumerate(wnames):
        wpack[:, i * P:(i + 1) * P] = consts[n]

    nc = bacc.Bacc("TRN2", target_bir_lowering=False, debug=False,
                   num_devices=B)
    x_d = nc.dram_tensor("x", [H, W], DT.float32, kind="ExternalInput").ap()
    p_d = nc.dram_tensor("prediction", [H, W], DT.int32,
                         kind="ExternalInput").ap()
    w_d = nc.dram_tensor("wpack", list(wpack.shape), DT.float32,
                         kind="ExternalInput").ap()
    y_d = nc.dram_tensor("y", [H, W], DT.float32, kind="ExternalOutput").ap()
    with tile.TileContext(nc) as tc:
        with ExitStack() as ctx:
            if K > 1:
                hint = (mybir.EngineType.PE, mybir.EngineType.DVE,
                        mybir.EngineType.Activation, mybir.EngineType.Pool,
                        mybir.EngineType.SP)
                with tc.For_i(0, K, 1, hint_engines=hint,
                              staggered_reset=True):
                    build_kernel(ctx, tc, [y_d], [x_d, p_d, w_d])
            else:
                build_kernel(ctx, tc, [y_d], [x_d, p_d, w_d])
    nc.compile()
    _CACHE[key] = nc
    _CACHE['wpack'] = wpack
    return nc, wpack


def _run(x, prediction, trace=False):
    from concourse.bass_utils import run_bass_kernel_spmd
    nc, wpack = _build_program()
    in_maps = []
    for i in range(B):
        in_maps.append({
            "x": np.ascontiguousarray(x[i, 0]).astype(np.float32),
            "prediction": np.ascontiguousarray(prediction[i, 0]).astype(np.int32),
            "wpack": wpack,
        })
    res = run_bass_kernel_spmd(nc, in_maps, core_ids=list(range(B)),
                               trace=trace)
    if trace:
        print(f"HW exec time: {res.exec_time_ns} ns "
              f"(mean {res.mean_exec_time_ns} ns, "
              f"slowest core {res.max_exec_time_core_id})")
        if res.instructions_and_trace:
            print("trace:", res.instructions_and_trace[1])
    out = np.stack([res.results[i]["y"] for i in range(B)], axis=0)
    return out[:, None, :, :].astype(np.float32)


def kernel(x: np.ndarray, prediction: np.ndarray) -> np.ndarray:
    return _run(x, prediction, trace=False)


def kernel_traced(x, prediction, trace=True):
    return _run(x, prediction, trace=trace)


if __name__ == "__main__":
    xs = np.random.randn(B, 1, H, W).astype(np.float32)
    ps = np.random.randint(0, 19, size=(B, 1, H, W)).astype(np.int32)
    print(kernel(xs, ps).shape)

